# revision 1
# baseline (speedup 1.0000x reference)
"""Trainium2 Bass kernel for the topopt compliance-loss problem.

Strategy (structured fast path):
  The reference's edofMat is the standard Q4 grid connectivity, so the
  gather U[:, edofMat] is a 2x2 node stencil over the displacement field
  viewed as a [513, 513, 2] node image.  Per element (x, y):
      ce[y, x] = u^T K u,   u = 8 DOFs of the 4 corner nodes
  With K = sym(KE) = V diag(lam) V^T (host eigh), ce = sum_r sign(lam_r)*G_r^2
  where G_r = a_r . u is a *linear* stencil -> computed on the TensorEngine
  as banded matmuls over the transposed node image (partitions = node rows,
  so the (dy, c) DOF offsets become partition offsets inside the matmul
  contraction window; the dx offset is a free-dim shift of the rhs).

  Per core: 2 batches (pure data parallel over B=16 on 8 cores).
  Dtypes: all matmuls run in float32r (4x cheaper than fp32 on the PE).
  The naive f32r W-rounding bias (~6e-5) is eliminated by quantizing the
  SOS vectors onto the measured f32r storage grid (round-to-nearest,
  12-bit mantissa) with error-compensating coordinate descent on the host,
  so the device-side convert is exact; U-data rounding is unbiased noise
  that averages out (~5e-6 end-to-end, verified on hardware).
  Device emits per-partition partial sums; host does the final O(B) scalars.

Fallback: any input not matching the structured grid (edofMat/penal/shape)
is computed on host in float64 numpy (same semantics as the reference).
"""

import sys

for _p in ('/opt/trn_rl_repo', '/opt/trn_rl_repo/concourse'):
    if _p not in sys.path:
        sys.path.insert(0, _p)

import numpy as np

B, NX, NY, NN = 16, 512, 512, 513
NDOF = 2 * NN * NN
NELE = NX * NY
N_CORES = 8
BPC = B // N_CORES  # batches per core
EMIN, EMAX = 1e-9, 1.0
DE = EMAX - EMIN

# edofMat column -> (dx, dy, c) node-stencil offsets (derived from the Q4
# connectivity: cols [2n1+2, 2n1+3, 2n2+2, 2n2+3, 2n2, 2n2+1, 2n1, 2n1+1])
COL_AX = (0, 0, 1, 1, 1, 1, 0, 0)
COL_AY = (1, 1, 1, 1, 0, 0, 0, 0)
COL_C = (0, 1, 0, 1, 0, 1, 0, 1)

N_PT = 11          # transposed-node-image tiles, partition stride 96
PT_W = 520         # free width (513 used)
N_YT = 4           # y-tiles of 128 per batch
# output partials column layout (per core, [128, 32]):
#   cols  i*8 + k        : compliance accumulation chain, batch i (8 links)
#   cols 16 + i*4 + yt   : rho partial sums
#   cols 24 + i*4 + yt   : vol partial sums
OUT_COLS = 32


def _build_edof():
    elx = np.repeat(np.arange(NX), NY)
    ely = np.tile(np.arange(NY), NX)
    n1 = (NY + 1) * elx + ely
    n2 = (NY + 1) * (elx + 1) + ely
    return np.stack([2 * n1 + 2, 2 * n1 + 3, 2 * n2 + 2, 2 * n2 + 3,
                     2 * n2, 2 * n2 + 1, 2 * n1, 2 * n1 + 1], axis=1)


def _build_consts(KE):
    """W0/W1 banded stencil matrices and the signed sum-selector S."""
    K = (KE.astype(np.float64) + KE.astype(np.float64).T) / 2
    lam, V = np.linalg.eigh(K)
    a = V * np.sqrt(np.abs(lam))[None, :]      # a[:, r]
    s = np.sign(lam)

    # Quantize the SOS vectors onto the f32r storage grid (round-to-nearest,
    # 12-bit mantissa — measured on device) with error-compensating
    # coordinate descent so sum_r s_r a_q a_q^T stays close to K.  The
    # device-side f32 -> f32r convert is then exact, letting the G-matmuls
    # run in f32r (4x cheaper than fp32) without the rounding bias.
    def _q12(v):
        m, e = np.frexp(np.float64(v))
        return np.round(m * 4096.0) / 4096.0 * 2.0 ** e

    def _ulp12(v):
        _, e = np.frexp(np.float64(v) if v != 0 else 1e-12)
        return 2.0 ** e / 4096.0

    aq = np.vectorize(_q12)(a)
    best = np.linalg.norm(K - (aq * s[None, :]) @ aq.T)
    for _ in range(40):
        improved = False
        for i in range(8):
            for r in range(8):
                v0 = aq[i, r]
                u = _ulp12(v0 if v0 != 0 else a[i, r])
                for k in (-3, -2, -1, 1, 2, 3):
                    aq[i, r] = v0 + k * u
                    n = np.linalg.norm(K - (aq * s[None, :]) @ aq.T)
                    if n < best - 1e-18:
                        best = n
                        v0 = aq[i, r]
                        improved = True
                aq[i, r] = v0
        if not improved:
            break
    a = aq
    W = np.zeros((2, 34, 128), np.float32)
    for r in range(8):
        for y16 in range(16):
            m = r * 16 + y16
            for i in range(8):
                W[COL_AX[i], 2 * y16 + 2 * COL_AY[i] + COL_C[i], m] += a[i, r]
    # 8 selector variants: S[:, j*128:(j+1)*128] maps SQ-pack rows (r, y16)
    # to ce rows 16j + y16 (zeros elsewhere; ce accumulates over j in PSUM)
    S = np.zeros((128, 8 * 128), np.float32)
    for j in range(8):
        for r in range(8):
            for y16 in range(16):
                S[r * 16 + y16, j * 128 + 16 * j + y16] = s[r]
    return W, S


def _numpy_fallback(rho, U, vol_field, solid_comp, KE, edofMat, penal, lambda_vol):
    rho64 = rho.astype(np.float64)
    U64 = U.astype(np.float64)
    Ue = U64[:, edofMat]                      # [B, nele, 8]
    ce = np.einsum('bei,ij,bej->be', Ue, KE.astype(np.float64), Ue)
    nb, nely, nelx = rho.shape
    ce = ce.reshape(nb, nelx, nely).transpose(0, 2, 1)
    compliance = ((EMIN + rho64 ** penal * (EMAX - EMIN)) * ce).sum(axis=(1, 2))
    n_ele = nelx * nely
    volfrac = vol_field.astype(np.float64).sum(axis=(1, 2)) / n_ele
    viol = np.abs(rho64.sum(axis=(1, 2)) / n_ele - volfrac)
    loss = compliance / solid_comp.astype(np.float64) + lambda_vol * viol
    return (loss.astype(np.float32), compliance.astype(np.float32),
            viol.astype(np.float32))


_NC_CACHE = {}


def _build_nc():
    if 'nc' in _NC_CACHE:
        return _NC_CACHE['nc']
    import os
    SKIP = set(os.environ.get('BASSK_SKIP', '').split(','))
    from contextlib import ExitStack
    from concourse import bass, mybir, tile

    f32 = mybir.dt.float32
    f32r = mybir.dt.float32r
    Copy = mybir.ActivationFunctionType.Copy
    nc = bass.Bass("TRN2", target_bir_lowering=False, debug=False)
    p_u = nc.declare_dram_parameter("u", [BPC, NN, 2 * NN], f32, isOutput=False)
    p_rho = nc.declare_dram_parameter("rho", [BPC, NY, NX], f32, isOutput=False)
    p_vol = nc.declare_dram_parameter("vol", [BPC, NY, NX], f32, isOutput=False)
    p_w = nc.declare_dram_parameter("wmat", [128, 768], f32, isOutput=False)
    p_s = nc.declare_dram_parameter("smat", [128, 1024], f32, isOutput=False)
    p_id = nc.declare_dram_parameter("ident", [128, 128], f32, isOutput=False)
    p_out = nc.declare_dram_parameter("partials", [128, OUT_COLS], f32,
                                      isOutput=True)

    with tile.TileContext(nc) as tc, ExitStack() as ctx:
        consts = ctx.enter_context(tc.tile_pool(name="consts", bufs=1))
        ni_p = ctx.enter_context(tc.tile_pool(name="ni", bufs=9))
        pt_p = ctx.enter_context(tc.tile_pool(name="pt", bufs=2 * N_PT + 2))
        sq_p = ctx.enter_context(tc.tile_pool(name="sq", bufs=6))
        fld_p = ctx.enter_context(tc.tile_pool(name="fld", bufs=4))
        ps_tr = ctx.enter_context(tc.tile_pool(name="pstr", bufs=2, space="PSUM"))
        ps_g = ctx.enter_context(tc.tile_pool(name="psg", bufs=3, space="PSUM"))
        ps_ce = ctx.enter_context(tc.tile_pool(name="psce", bufs=2, space="PSUM"))
        ps_tc = ctx.enter_context(tc.tile_pool(name="pstc", bufs=1, space="PSUM"))

        wmat_f = consts.tile([128, 768], f32)
        wmat = consts.tile([128, 768], f32r)
        smat_f = consts.tile([128, 1024], f32)
        smat = consts.tile([128, 1024], f32r)
        ident = consts.tile([128, 128], f32)
        out_t = consts.tile([128, OUT_COLS], f32)
        zero_c = consts.tile([128, 1], f32)
        junk = consts.tile([128, 512], f32)
        nc.sync.dma_start(out=wmat_f[:], in_=p_w[:])
        nc.vector.tensor_copy(out=wmat[:], in_=wmat_f[:])
        nc.sync.dma_start(out=smat_f[:], in_=p_s[:])
        nc.vector.tensor_copy(out=smat[:], in_=smat_f[:])
        nc.sync.dma_start(out=ident[:], in_=p_id[:])
        nc.vector.memset(zero_c[:], 0.0)

        all_pt = {}
        for bi in range(BPC):
            # ---- build PT tiles (transposed node image) ----
            ni_tiles = []
            for xc in range(4):
                t = ni_p.tile([128, 2 * NN], f32, tag="ni")
                nc.sync.dma_start(out=t[:], in_=p_u[bi, 128 * xc:128 * (xc + 1), :])
                ni_tiles.append(t)
            ni4 = ni_p.tile([1, 2 * NN], f32, tag="ni4")
            nc.sync.dma_start(out=ni4[:], in_=p_u[bi, NN - 1:NN, :])

            pt_tiles = []
            for t_i in range(0 if 'nopt' in SKIP else N_PT):
                w = min(128, 2 * NN - 96 * t_i)
                pt = pt_p.tile([128, PT_W], f32r, tag="pt")
                if w < 128 and 'memset' not in SKIP:
                    nc.gpsimd.memset(pt[:].bitcast(f32), 0.0)
                stage = ps_tr.tile([128, 512], f32, tag="pstr")
                for xc in range(4):
                    nc.tensor.transpose(
                        out=stage[:w, 128 * xc:128 * (xc + 1)],
                        in_=ni_tiles[xc][:, 96 * t_i:96 * t_i + w],
                        identity=ident[:],
                    )
                if t_i % 2 == 0:
                    nc.vector.tensor_copy(out=pt[:w, 0:512], in_=stage[:w, :])
                else:
                    nc.scalar.copy(out=pt[:w, 0:512], in_=stage[:w, :])
                # last node-column (x = 512) via a tiny transpose
                tinyps = ps_tc.tile([128, 1], f32, tag="pstc")
                nc.tensor.transpose(
                    out=tinyps[:w, 0:1],
                    in_=ni4[0:1, 96 * t_i:96 * t_i + w],
                    identity=ident[0:1, 0:1],
                )
                nc.vector.tensor_copy(out=pt[:w, 512:513], in_=tinyps[:w, 0:1])
                pt_tiles.append(pt)
            all_pt[bi] = pt_tiles

        for bi in range(BPC):
            pt_tiles = all_pt[bi]
            # ---- per y-tile: weights field, sums, stencil matmuls ----
            for yt in range(N_YT):
                ysl = slice(128 * yt, 128 * (yt + 1))
                if 'nofld' in SKIP:
                    rt = fld_p.tile([128, 512], f32, tag="rho")
                else:
                    rt = fld_p.tile([128, 512], f32, tag="rho")
                vt = fld_p.tile([128, 512], f32, tag="vol")
                nc.sync.dma_start(out=rt[:], in_=p_rho[bi, ysl, :])
                nc.sync.dma_start(out=vt[:], in_=p_vol[bi, ysl, :])
                r2 = fld_p.tile([128, 512], f32, tag="r2")
                r3 = fld_p.tile([128, 512], f32, tag="r3")
                wt = fld_p.tile([128, 512], f32, tag="wt")
                nc.scalar.square(out=r2[:], in_=rt[:])
                if 'gmul' in SKIP:
                    nc.vector.tensor_mul(r3[:], r2[:], rt[:])
                else:
                    nc.gpsimd.tensor_mul(r3[:], r2[:], rt[:])
                # w = EMIN + DE * rho^3
                nc.scalar.activation(wt[:], r3[:], Copy, bias=EMIN, scale=DE)
                # partial sums of rho and vol (over x) via ACT accumulators
                nc.scalar.activation(
                    junk[:], rt[:], Copy, bias=0.0, scale=1.0,
                    accum_out=out_t[:, 16 + bi * 4 + yt: 17 + bi * 4 + yt])
                nc.scalar.activation(
                    junk[:], vt[:], Copy, bias=0.0, scale=1.0,
                    accum_out=out_t[:, 24 + bi * 4 + yt: 25 + bi * 4 + yt])

                if 'nog' in SKIP:
                    continue
                ce = ps_ce.tile([128, 512], f32, tag="psce", name="ce")
                for j in range(8):
                    mi = 8 * yt + j
                    t_i = mi // 3
                    g = ps_g.tile([128, 512], f32, tag="psg")
                    v = mi % 3
                    nc.tensor.matmul(
                        out=g[:],
                        lhsT=wmat[:, v * 256: v * 256 + 128],
                        rhs=pt_tiles[t_i][:, 0:512],
                        start=True, stop=False)
                    nc.tensor.matmul(
                        out=g[:],
                        lhsT=wmat[:, v * 256 + 128: v * 256 + 256],
                        rhs=pt_tiles[t_i][:, 1:513],
                        start=False, stop=True)
                    if 'nosq' in SKIP:
                        continue
                    sq = sq_p.tile([128, 512], f32r, tag="sq")
                    nc.scalar.square(out=sq[:], in_=g[:])
                    # selector j places this pack's rows at 16j + y16;
                    # all 8 packs accumulate into one full-height ce tile
                    nc.tensor.matmul(
                        out=ce[:],
                        lhsT=smat[:, j * 128:(j + 1) * 128],
                        rhs=sq[:],
                        start=(j == 0), stop=(j == 7))
                # weighted reduce; each y-tile gets its own output column
                # (host sums the 4 columns per batch)
                scratch = fld_p.tile([128, 512], f32, tag="scr")
                nc.vector.scalar_tensor_tensor(
                    out=scratch[:],
                    in0=ce[:],
                    scalar=1.0,
                    in1=wt[:],
                    op0=mybir.AluOpType.mult,
                    op1=mybir.AluOpType.mult,
                    accum_out=out_t[:, bi * 4 + yt: bi * 4 + yt + 1])

        nc.sync.dma_start(out=p_out[:], in_=out_t[:])

    # walrus in this container rejects >1 sem-wait per instruction; split.
    _split_waits(nc)
    _NC_CACHE['nc'] = nc
    return nc


def _split_waits(nc):
    from concourse import mybir
    drainable = {"PE", "DVE", "Activation", "Pool", "SP"}
    n = 0
    for f in nc.m.functions:
        for bb in f.blocks:
            insts = list(bb.instructions)
            new_list = []
            changed = False
            for ins in insts:
                si = ins.sync_info
                waits = list(si.on_wait) if si is not None and si.on_wait else []
                eng = str(ins.engine).split(".")[-1]
                if len(waits) > 1 and eng in drainable:
                    changed = True
                    for w in waits[:-1]:
                        d = mybir.InstDrain(name=f"{ins.name}-ws{n}", ins=[], outs=[])
                        d.engine = ins.engine
                        d.sync_info = mybir.SyncInfo(on_wait=[w], on_update=[])
                        new_list.append(d)
                        n += 1
                    ins.sync_info = mybir.SyncInfo(
                        on_wait=[waits[-1]],
                        on_update=list(si.on_update) if si.on_update else [])
                new_list.append(ins)
            if changed:
                bb.instructions = new_list
    return n


def kernel(rho, U, vol_field, solid_comp, KE, edofMat, penal, lambda_vol):
    rho = np.asarray(rho, np.float32)
    U = np.asarray(U, np.float32)
    vol = np.asarray(vol_field, np.float32)
    sc = np.asarray(solid_comp, np.float32)
    KEn = np.asarray(KE, np.float32)
    ed = np.asarray(edofMat)
    pen = int(np.asarray(penal))
    lv = float(np.asarray(lambda_vol))

    structured = (
        rho.shape == (B, NY, NX) and U.shape == (B, NDOF)
        and vol.shape == (B, NY, NX) and ed.shape == (NELE, 8)
        and pen == 3
        and np.array_equal(ed.astype(np.int64), _build_edof())
    )
    if not structured:
        return _numpy_fallback(rho, U, vol, sc, KEn,
                               ed.astype(np.int64), pen, lv)

    from concourse.bass_utils import run_bass_kernel_spmd

    W, S = _build_consts(KEn)
    wmat = np.zeros((128, 768), np.float32)
    for v in range(3):
        for ax in range(2):
            wmat[32 * v:32 * v + 34, v * 256 + ax * 128: v * 256 + (ax + 1) * 128] = W[ax]
    ident = np.eye(128, dtype=np.float32)

    nc = _build_nc()
    in_maps = []
    for c in range(N_CORES):
        bsl = slice(BPC * c, BPC * (c + 1))
        in_maps.append({
            "u": np.ascontiguousarray(U[bsl].reshape(BPC, NN, 2 * NN)),
            "rho": np.ascontiguousarray(rho[bsl]),
            "vol": np.ascontiguousarray(vol[bsl]),
            "wmat": wmat,
            "smat": S,
            "ident": ident,
        })
    res = run_bass_kernel_spmd(nc, in_maps, list(range(N_CORES)))
    _NC_CACHE['last_result'] = res

    compliance = np.zeros(B, np.float64)
    rho_sum = np.zeros(B, np.float64)
    vol_sum = np.zeros(B, np.float64)
    for c in range(N_CORES):
        p = res.results[c]["partials"].astype(np.float64)
        for i in range(BPC):
            b = BPC * c + i
            compliance[b] = p[:, i * 4: i * 4 + 4].sum()
            rho_sum[b] = p[:, 16 + i * 4: 16 + i * 4 + 4].sum()
            vol_sum[b] = p[:, 24 + i * 4: 24 + i * 4 + 4].sum()
    volfrac = vol_sum / NELE
    viol = np.abs(rho_sum / NELE - volfrac)
    loss = compliance / sc.astype(np.float64) + lv * viol
    return (loss.astype(np.float32), compliance.astype(np.float32),
            viol.astype(np.float32))



# revision 11
# speedup vs baseline: 2.8470x; 2.8470x over previous
"""Trainium2 Bass kernel for the topopt compliance-loss problem (fp8 DoubleRow).

Math: the reference's edofMat is the standard Q4 grid connectivity, so
U[:, edofMat] is a 2x2 node stencil over the displacement field viewed as a
[513, 513, 2] node image.  With K = sym(KE) = V diag(lam) V^T:
    ce[y, x] = sum_r s_r * G_r^2,   G_r = a_r . u  (linear 8-tap stencil)
Device pipeline per 128x512 y-tile (output rows packed (8r x 16y)):
  - G-pack: TWO fp8 DoubleRow matmuls (hi + lo split of the stencil weights,
    accumulated in PSUM f32).  The DoubleRow pair dim carries the dx=0/dx=1
    node columns, so one matmul applies all 8 taps; fp8 weight-quantization
    error is killed by the hi+lo split plus error-compensating coordinate
    descent on the host (K reconstruction err ~1e-4).
  - square G -> fp8 (round-robin across Activation/Vector/GpSimd engines)
  - selector: ONE fp8 DoubleRow matmul folds TWO sq-packs into ce rows
    (scatter (r,y16)->y with the eigenvalue signs; +-1 is exact in fp8)
  - compliance: scalar_tensor_tensor(ce * rho^3) with accum_out column
    (EMIN * sum(ce) term is dropped: it is ~1e-9 relative)
  - rho^3 chain and the sum(rho - vol) accumulation run in bf16 on the
    Vector engine (random-rounding noise ~1e-6 relative after the sums)
Host side: U is pretransposed to node-dof-major layout, windowed per pack,
fp8-quantized, and laid out so each batch's U/rho/vol is ONE contiguous DMA
(HWDGE descriptor-generation costs ~630ns per dma_start, so DMA count is 9).
Data parallel over B=16 on 8 cores (2 batches each); host does the final
O(B) scalar assembly in float64.

Fallback: any input not matching the structured grid (edofMat/penal/shape)
is computed on host in float64 numpy (same semantics as the reference).
"""

import sys

for _p in ('/opt/trn_rl_repo', '/opt/trn_rl_repo/concourse'):
    if _p not in sys.path:
        sys.path.insert(0, _p)

import numpy as np
import ml_dtypes

F8NP = ml_dtypes.float8_e4m3
BF16NP = ml_dtypes.bfloat16

B, NX, NY, NN = 16, 512, 512, 513
NDOF = 2 * NN * NN
NELE = NX * NY
N_CORES = 8
BPC = B // N_CORES  # batches per core
EMIN, EMAX = 1e-9, 1.0
DE = EMAX - EMIN
N_YT = 4            # y-tiles of 128 per batch
N_PACK = 32         # (8r x 16y) packs per batch = 8 per y-tile

# edofMat column -> (dx, dy, c) node-stencil offsets (derived from the Q4
# connectivity: cols [2n1+2, 2n1+3, 2n2+2, 2n2+3, 2n2, 2n2+1, 2n1, 2n1+1])
COL_AX = (0, 0, 1, 1, 1, 1, 0, 0)
COL_AY = (1, 1, 1, 1, 0, 0, 0, 0)
COL_C = (0, 1, 0, 1, 0, 1, 0, 1)

# per-y-tile engine pattern for the 8 G-squares: A=Activation, D=Vector(DVE),
# P=GpSimd(Pool).  Balanced against each engine's fixed per-y-tile load.
# per-y-tile engine for each of the 4 sq-pack PAIRS: 'A' = Activation squares
# (fp8 + DoubleRow selector), 'D' = DVE copy->bf16 + bf16 square (+ 2 plain
# bf16 selector matmuls).  GPSIMD cannot touch PSUM; DVE cannot read the same
# PSUM tile twice, hence the copy.
PAIR_ENG = (('A', 'A', 'D', 'A'), ('A', 'A', 'D', 'A'),
            ('A', 'A', 'D', 'A'), ('A', 'A', 'D', 'A'),
            ('A', 'A', 'D', 'A'), ('A', 'A', 'D', 'A'),
            ('A', 'A', 'D', 'A'), ('A', 'D', 'A', 'D'))

OUT_COLS = 16  # cols bi*8+yt: sum(rho^3*ce); bi*8+4+yt: sum(rho-vol)


def _build_edof():
    elx = np.repeat(np.arange(NX), NY)
    ely = np.tile(np.arange(NY), NX)
    n1 = (NY + 1) * elx + ely
    n2 = (NY + 1) * (elx + 1) + ely
    return np.stack([2 * n1 + 2, 2 * n1 + 3, 2 * n2 + 2, 2 * n2 + 3,
                     2 * n2, 2 * n2 + 1, 2 * n1, 2 * n1 + 1], axis=1)


def _q8(v):
    return np.asarray(v).astype(F8NP).astype(np.float64)


def _ulp8(v):
    av = abs(float(v))
    if av < 2.0 ** -9:
        return 2.0 ** -9
    _, e = np.frexp(av)
    return max(2.0 ** (int(e) - 4), 2.0 ** -9)


def _build_consts(KE):
    """fp8 hi/lo stencil weights and selector matrices.

    Returns (w8 [34, 512] fp8: [whiA|whiB|wloA|wloB] 128-col blocks,
             sel8 [128, 1024] fp8: 4 pair-selectors [selA|selB] each)
    """
    K = (KE.astype(np.float64) + KE.astype(np.float64).T) / 2
    lam, V = np.linalg.eigh(K)
    a = V * np.sqrt(np.abs(lam))[None, :]      # a[:, r]
    s = np.sign(lam)
    s[s == 0] = 1.0

    hi = _q8(a)
    lo = _q8(a - hi)

    # error-compensating coordinate descent on the lo entries so that
    # sum_r s_r w w^T (w = hi+lo on the fp8-pair grid) stays close to K
    def kerr(h, l):
        w = h + l
        return np.linalg.norm(K - (w * s[None, :]) @ w.T)

    best = kerr(hi, lo)
    for _ in range(30):
        improved = False
        for i in range(8):
            for r in range(8):
                v0 = lo[i, r]
                u = _ulp8(v0 if v0 != 0 else max(abs(a[i, r]) * 2 ** -4, 2 ** -9))
                for k in (-3, -2, -1, 1, 2, 3):
                    lo[i, r] = _q8(v0 + k * u)
                    n = kerr(hi, lo)
                    if n < best - 1e-15:
                        best = n
                        v0 = lo[i, r]
                        improved = True
                lo[i, r] = v0
        if not improved:
            break

    # scatter into the [34, 128] banded stencil layout, ax 0/1 = dx blocks
    Whi = np.zeros((2, 34, 128), np.float64)
    Wlo = np.zeros((2, 34, 128), np.float64)
    for r in range(8):
        for y16 in range(16):
            m = r * 16 + y16
            for i in range(8):
                row = 2 * y16 + 2 * COL_AY[i] + COL_C[i]
                Whi[COL_AX[i], row, m] += hi[i, r]
                Wlo[COL_AX[i], row, m] += lo[i, r]
    w8 = np.zeros((34, 512), np.float64)
    w8[:, 0:128] = Whi[0]
    w8[:, 128:256] = Whi[1]
    w8[:, 256:384] = Wlo[0]
    w8[:, 384:512] = Wlo[1]

    # 4 pair-selectors: pair jp folds packs j=2jp (A) and j=2jp+1 (B) into
    # ce rows 16j + y16 with sign s_r
    sel8 = np.zeros((128, 1024), np.float64)
    for jp in range(4):
        for half in range(2):
            j = 2 * jp + half
            for r in range(8):
                for y16 in range(16):
                    sel8[r * 16 + y16, 256 * jp + 128 * half + 16 * j + y16] = s[r]
    return w8.astype(F8NP), sel8.astype(F8NP)


def _numpy_fallback(rho, U, vol_field, solid_comp, KE, edofMat, penal, lambda_vol):
    rho64 = rho.astype(np.float64)
    U64 = U.astype(np.float64)
    Ue = U64[:, edofMat]                      # [B, nele, 8]
    ce = np.einsum('bei,ij,bej->be', Ue, KE.astype(np.float64), Ue)
    nb, nely, nelx = rho.shape
    ce = ce.reshape(nb, nelx, nely).transpose(0, 2, 1)
    compliance = ((EMIN + rho64 ** penal * (EMAX - EMIN)) * ce).sum(axis=(1, 2))
    n_ele = nelx * nely
    volfrac = vol_field.astype(np.float64).sum(axis=(1, 2)) / n_ele
    viol = np.abs(rho64.sum(axis=(1, 2)) / n_ele - volfrac)
    loss = compliance / solid_comp.astype(np.float64) + lambda_vol * viol
    return (loss.astype(np.float32), compliance.astype(np.float32),
            viol.astype(np.float32))


_NC_CACHE = {}


def _build_nc():
    if 'nc' in _NC_CACHE:
        return _NC_CACHE['nc']
    from contextlib import ExitStack
    from concourse import bass, mybir, tile

    f32 = mybir.dt.float32
    bf16 = mybir.dt.bfloat16
    f8 = mybir.dt.float8e4
    DR = mybir.MatmulPerfMode.DoubleRow
    MULT = mybir.AluOpType.mult
    SUB = mybir.AluOpType.subtract

    nc = bass.Bass("TRN2", target_bir_lowering=False, debug=False)
    # u8: per batch [68 part, 16 win-groups, 1024]: window mi at
    # [34*(mi//16) : +34, 1024*(mi%16) : +1024]; cols 0:512 = dx0, 512:1024 = dx1
    p_u8 = nc.declare_dram_parameter("u8", [BPC, 68, 16, 1024], f8, isOutput=False)
    # rho/vol: per batch [128 part, 4 y-tiles, 512] bf16
    p_rho = nc.declare_dram_parameter("rho", [BPC, 128, N_YT, 512], f32,
                                      isOutput=False)
    p_vol = nc.declare_dram_parameter("vol", [BPC, 128, N_YT, 512], f32,
                                      isOutput=False)
    p_w8 = nc.declare_dram_parameter("w8", [34, 512], f8, isOutput=False)
    p_sel = nc.declare_dram_parameter("sel8", [128, 1024], f8, isOutput=False)
    p_selb = nc.declare_dram_parameter("selb", [128, 1024], bf16, isOutput=False)
    p_out = nc.declare_dram_parameter("partials", [128, OUT_COLS], f32,
                                      isOutput=True)

    with tile.TileContext(nc) as tc, ExitStack() as ctx:
        consts = ctx.enter_context(tc.tile_pool(name="consts", bufs=1))
        u_p = ctx.enter_context(tc.tile_pool(name="u", bufs=2))
        fld_p = ctx.enter_context(tc.tile_pool(name="fld", bufs=2))
        r_p = ctx.enter_context(tc.tile_pool(name="r", bufs=8))
        jk_p = ctx.enter_context(tc.tile_pool(name="jk", bufs=6))
        gc_p = ctx.enter_context(tc.tile_pool(name="gc", bufs=6))
        sq_p = ctx.enter_context(tc.tile_pool(name="sq", bufs=8))
        ps_g = ctx.enter_context(tc.tile_pool(name="psg", bufs=6, space="PSUM"))
        ps_ce = ctx.enter_context(tc.tile_pool(name="psce", bufs=2, space="PSUM"))

        w8 = consts.tile([98, 512], f8)
        sel = consts.tile([128, 1024], f8)
        selb = consts.tile([128, 1024], bf16)
        out_t = consts.tile([128, OUT_COLS], f32)
        # W replicated at partitions 0 and 64: lhsT must share the rhs base
        nc.sync.dma_start(out=w8[0:34, :], in_=p_w8[:])
        nc.sync.dma_start(out=w8[64:98, :], in_=p_w8[:])
        nc.sync.dma_start(out=sel[:], in_=p_sel[:])
        nc.sync.dma_start(out=selb[:], in_=p_selb[:])

        whi = [w8[64 * h:64 * h + 34, 0:256].rearrange("p (a b) -> p a b", a=2)
               for h in range(2)]
        wlo = [w8[64 * h:64 * h + 34, 256:512].rearrange("p (a b) -> p a b", a=2)
               for h in range(2)]

        u_tiles, rho_tiles, vol_tiles = [], [], []
        for bi in range(BPC):
            # window group h=1 sits at partition 64 (matmul base-partition rule)
            ut = u_p.tile([98, 16384], f8, tag="u")
            src8 = p_u8[bi].rearrange("p a b -> p (a b)")
            nc.sync.dma_start(out=ut[0:34, 0:8192], in_=src8[0:34, 0:8192])
            nc.sync.dma_start(out=ut[0:34, 8192:16384], in_=src8[0:34, 8192:16384])
            nc.sync.dma_start(out=ut[64:98, 0:8192], in_=src8[34:68, 0:8192])
            nc.sync.dma_start(out=ut[64:98, 8192:16384], in_=src8[34:68, 8192:16384])
            rt = fld_p.tile([128, N_YT * 512], f32, tag="rho")
            nc.sync.dma_start(out=rt[:], in_=p_rho[bi].rearrange("p a b -> p (a b)"))
            vt = fld_p.tile([128, N_YT * 512], f32, tag="vol")
            nc.sync.dma_start(out=vt[:], in_=p_vol[bi].rearrange("p a b -> p (a b)"))
            u_tiles.append(ut)
            rho_tiles.append(rt)
            vol_tiles.append(vt)

        for bi in range(BPC):
            ut, rt, vt = u_tiles[bi], rho_tiles[bi], vol_tiles[bi]
            for yt in range(N_YT):
                xsl = slice(512 * yt, 512 * (yt + 1))
                rho_y = rt[:, xsl]
                vol_y = vt[:, xsl]
                # rho^3 in bf16 on DVE
                r2 = r_p.tile([128, 512], f32, tag="r2")
                r3 = r_p.tile([128, 512], bf16, tag="r3")
                nc.gpsimd.tensor_tensor(out=r2[:], in0=rho_y, in1=rho_y, op=MULT)
                nc.gpsimd.tensor_tensor(out=r3[:], in0=r2[:], in1=rho_y, op=MULT)
                # sum(rho - vol) accumulation
                junk = jk_p.tile([128, 512], f32, tag="jk")
                nc.vector.scalar_tensor_tensor(
                    out=junk[:], in0=rho_y, scalar=1.0, in1=vol_y,
                    op0=MULT, op1=SUB,
                    accum_out=out_t[:, bi * 8 + 4 + yt: bi * 8 + 5 + yt])

                ce = ps_ce.tile([128, 512], f32, tag="ce")
                pairs = PAIR_ENG[2 * bi * N_YT // 2 + yt]
                for jp in range(4):
                    eng = pairs[jp]
                    if eng == 'A':
                        sqt = sq_p.tile([128, 1024], f8, tag="sq")
                    else:
                        sqt = sq_p.tile([128, 1024], bf16, tag="sqb")
                    for half_i in range(2):
                        j = 2 * jp + half_i
                        mi = 8 * yt + j
                        h, gcol = mi // 16, mi % 16
                        win = ut[64 * h:64 * h + 34,
                                 1024 * gcol:1024 * gcol + 1024]
                        winp = win.rearrange("p (a b) -> p a b", a=2)
                        g = ps_g.tile([128, 512], f32, tag="g")
                        nc.tensor.matmul(out=g[:], lhsT=whi[h], rhs=winp,
                                         perf_mode=DR, start=True, stop=False)
                        nc.tensor.matmul(out=g[:], lhsT=wlo[h], rhs=winp,
                                         perf_mode=DR, start=False, stop=True)
                        half = sqt[:, 512 * half_i:512 * half_i + 512]
                        if eng == 'A':
                            nc.scalar.square(out=half, in_=g[:])
                        else:
                            gc = gc_p.tile([128, 512], bf16, tag="gc")
                            nc.vector.tensor_copy(out=gc[:], in_=g[:])
                            nc.vector.tensor_tensor(out=half, in0=gc[:],
                                                    in1=gc[:], op=MULT)
                    if eng == 'A':
                        selp = sel[:, 256 * jp:256 * jp + 256].rearrange(
                            "p (a b) -> p a b", a=2)
                        sqp = sqt[:, 0:1024].rearrange("p (a b) -> p a b", a=2)
                        nc.tensor.matmul(out=ce[:], lhsT=selp, rhs=sqp,
                                         perf_mode=DR,
                                         start=(jp == 0), stop=(jp == 3))
                    else:
                        for half_i in range(2):
                            nc.tensor.matmul(
                                out=ce[:],
                                lhsT=selb[:, 256 * jp + 128 * half_i:
                                          256 * jp + 128 * half_i + 128],
                                rhs=sqt[:, 512 * half_i:512 * half_i + 512],
                                start=(jp == 0 and half_i == 0),
                                stop=(jp == 3 and half_i == 1))
                # compliance accumulation: sum(ce * rho^3) on GpSimd
                junk32 = jk_p.tile([128, 512], f32, tag="jk32")
                nc.vector.scalar_tensor_tensor(
                    out=junk32[:], in0=ce[:], scalar=1.0, in1=r3[:],
                    op0=MULT, op1=MULT,
                    accum_out=out_t[:, bi * 8 + yt: bi * 8 + 1 + yt])

        nc.sync.dma_start(out=p_out[:], in_=out_t[:])

    # walrus in this container rejects >1 sem-wait per instruction; split.
    _split_waits(nc)
    _NC_CACHE['nc'] = nc
    return nc


def _split_waits(nc):
    from concourse import mybir
    drainable = {"PE", "DVE", "Activation", "Pool", "SP"}
    n = 0
    for f in nc.m.functions:
        for bb in f.blocks:
            insts = list(bb.instructions)
            new_list = []
            changed = False
            for ins in insts:
                si = ins.sync_info
                waits = list(si.on_wait) if si is not None and si.on_wait else []
                eng = str(ins.engine).split(".")[-1]
                if len(waits) > 1 and eng in drainable:
                    changed = True
                    for w in waits[:-1]:
                        d = mybir.InstDrain(name=f"{ins.name}-ws{n}", ins=[], outs=[])
                        d.engine = ins.engine
                        d.sync_info = mybir.SyncInfo(on_wait=[w], on_update=[])
                        new_list.append(d)
                        n += 1
                    ins.sync_info = mybir.SyncInfo(
                        on_wait=[waits[-1]],
                        on_update=list(si.on_update) if si.on_update else [])
                new_list.append(ins)
            if changed:
                bb.instructions = new_list
    return n


def _prep_u8(Ub):
    """[BPC, NDOF] f32 -> [BPC, 68, 16, 1024] fp8 pair-blocked windows."""
    out = np.empty((BPC, 68, 16, 1024), F8NP)
    rows = 32 * np.arange(N_PACK)[:, None] + np.arange(34)[None, :]
    for i in range(BPC):
        ut8 = np.ascontiguousarray(
            Ub[i].reshape(NN, 2 * NN).T).astype(F8NP)   # [1026, 513]
        wins = ut8[rows]                                # [32, 34, 513]
        for h in range(2):
            blk = wins[16 * h:16 * h + 16]              # [16, 34, 513]
            t = blk.transpose(1, 0, 2)                  # [34, 16, 513]
            out[i, 34 * h:34 * h + 34, :, 0:512] = t[:, :, 0:512]
            out[i, 34 * h:34 * h + 34, :, 512:1024] = t[:, :, 1:513]
    return out


def _prep_field(fb):
    """[BPC, NY, NX] f32 -> [BPC, 128, N_YT, 512] f32 (partition-major).

    f32 (not bf16): viol = |sum(rho - vol)|/n is a near-total cancellation,
    so the field sums need full input precision."""
    return np.ascontiguousarray(
        fb.reshape(BPC, N_YT, 128, NX).transpose(0, 2, 1, 3).astype(np.float32))


def kernel(rho, U, vol_field, solid_comp, KE, edofMat, penal, lambda_vol):
    rho = np.asarray(rho, np.float32)
    U = np.asarray(U, np.float32)
    vol = np.asarray(vol_field, np.float32)
    sc = np.asarray(solid_comp, np.float32)
    KEn = np.asarray(KE, np.float32)
    ed = np.asarray(edofMat)
    pen = int(np.asarray(penal))
    lv = float(np.asarray(lambda_vol))

    structured = (
        rho.shape == (B, NY, NX) and U.shape == (B, NDOF)
        and vol.shape == (B, NY, NX) and ed.shape == (NELE, 8)
        and pen == 3
        and np.array_equal(ed.astype(np.int64), _build_edof())
    )
    if not structured:
        return _numpy_fallback(rho, U, vol, sc, KEn,
                               ed.astype(np.int64), pen, lv)

    from concourse.bass_utils import run_bass_kernel_spmd

    w8, sel8 = _build_consts(KEn)
    nc = _build_nc()
    in_maps = []
    for c in range(N_CORES):
        bsl = slice(BPC * c, BPC * (c + 1))
        in_maps.append({
            "u8": _prep_u8(U[bsl]),
            "rho": _prep_field(rho[bsl]),
            "vol": _prep_field(vol[bsl]),
            "w8": w8,
            "sel8": sel8,
            "selb": sel8.astype(np.float32).astype(BF16NP),
        })
    res = run_bass_kernel_spmd(nc, in_maps, list(range(N_CORES)))
    _NC_CACHE['last_result'] = res

    compliance = np.zeros(B, np.float64)
    dsum = np.zeros(B, np.float64)
    for c in range(N_CORES):
        p = res.results[c]["partials"].astype(np.float64)
        for i in range(BPC):
            b = BPC * c + i
            compliance[b] = DE * p[:, i * 8: i * 8 + 4].sum()
            dsum[b] = p[:, i * 8 + 4: i * 8 + 8].sum()
    viol = np.abs(dsum) / NELE
    loss = compliance / sc.astype(np.float64) + lv * viol
    return (loss.astype(np.float32), compliance.astype(np.float32),
            viol.astype(np.float32))
